# revision 22
# baseline (speedup 1.0000x reference)
"""AsymAttentionLayer Trainium2 kernel — data-parallel over B on 8 NeuronCores.

Reference computation (per batch element b, NUM_G=32, g=32, num_p=8, dim=512,
H=8, E=64):
  stage1: attention within groups of g=32 tokens (seq=(ng,p), pos=gg)
  stage2: attention across groups (seq=(gg,p), pos=ng)
  7 linears of [tokens,512]x[512,512].

Device-side layout: feature-major activations xT [dim, token]; P-MAJOR token
order t1 = p*1024 + ng*32 + gg.  The inter-stage ng<->gg transpose preserves
p, so stage-2 chunk (p, gg-half) depends only on the two stage-1 chunks with
the same p — stage-1 and stage-2 chunks are software-pipelined in ONE
interleaved stream (no stage barrier), and the t1->t2 scatter (GpSimd, off
the critical engines, 16-element runs) is spread over the whole kernel.

Per step the PE alternates dense linear streams with SHORT small-matmul
bursts (one scores quarter / one AV quarter at a time): the small bursts'
LDWEIGHTS traffic hides under the dense streams' rhs-streaming windows, and
the PE never idles long enough for the HAM clock gate to re-throttle.
All matmuls bf16 with f32 PSUM accumulation.
"""

import os
import sys

import numpy as np

sys.path.insert(0, "/opt/trn_rl_repo")

NUM_G = 32
G = 32
NUM_P = 8
B = 8
D = 512
H = 8
E = 64
T = NUM_G * G * NUM_P  # 8192 tokens per core
CH = 512  # tokens per chunk
NCHUNK = T // CH  # 16
NSEQ = CH // G  # 16 sequences per chunk

# Attention weight-load grouping: how many [64x32] q-blocks (scores) /
# [32x64] v-blocks (AV) share one LDWEIGHTS.  1 = one LDW per matmul.
GROUP_S = int(os.environ.get("KB_GROUP_S", "1"))
GROUP_AV = int(os.environ.get("KB_GROUP_AV", "1"))

_GRAPH = None
LAST_EXEC_TIME_NS = None
LAST_TRACE = None

# chunk ids: 0..15 stage-1 (c: p=c//2, ng half c%2), 16..31 stage-2
# (j=c-16: p=j//2, gg half j%2).  Stage-2 chunk j needs stage-1 chunks
# 2*(j//2) and 2*(j//2)+1 scattered into y1.
ORDER = [0, 1, 2, 3, 4,
         16, 5, 17, 6, 18, 7, 19, 8, 20, 9, 21, 10, 22, 11, 23, 12,
         24, 13, 25, 14, 26, 15, 27,
         28, 29, 30, 31]


def _build_graph():
    import concourse.bass as bass
    from concourse import bacc, mybir
    from concourse.tile import TileContext

    f32 = mybir.dt.float32
    bf16 = mybir.dt.bfloat16
    AF = mybir.ActivationFunctionType
    ALU = mybir.AluOpType
    AX = mybir.AxisListType

    nc = bacc.Bacc()

    # group-LDW integrity tracking (verified post-finalize)
    grp_mm = {}
    grp_ldw = {}
    kept_ldw = {}

    def _ins(r):
        return getattr(r, "ins", r)

    import concourse.tile as _tile_mod

    def _ldw_region(ins):
        tp = ins.tile_position or (0, 0)
        ts = ins.tile_size or (128, 128)
        return (tp[0], tp[0] + ts[0], tp[1], tp[1] + ts[1])

    def _overlap(a, b):
        return a[0] < b[1] and b[0] < a[1] and a[2] < b[3] and b[2] < a[3]

    def _legalize_filter(orig):
        def wrapped(ordered, nc_, *a, **kw):
            out = orig(ordered, nc_, *a, **kw)
            n_drop = n_keep = 0
            for bb in list(out.keys()):
                insts = out[bb]
                ins_ldw_mm = {}
                for i, ins in enumerate(insts[:-1]):
                    nxt = insts[i + 1]
                    if (
                        isinstance(ins, mybir.InstLdweights)
                        and ins.name not in grp_ldw
                        and isinstance(nxt, mybir.InstMatmult)
                        and nxt.name in grp_mm
                    ):
                        ins_ldw_mm[ins.name] = nxt.name
                if not ins_ldw_mm:
                    continue
                ldw_events = [
                    (i, ins.name,
                     grp_ldw.get(ins.name) or _ldw_region(ins))
                    for i, ins in enumerate(insts)
                    if isinstance(ins, mybir.InstLdweights)
                ]
                pos = {ins.name: i for i, ins in enumerate(insts)}
                drop = set()
                for i, ins in enumerate(insts):
                    if not (isinstance(ins, mybir.InstMatmult) and ins.name in grp_mm):
                        continue
                    g, reg = grp_mm[ins.name]
                    gp = pos.get(g)
                    if gp is None or gp > i:
                        continue
                    clean = True
                    for li, ln, lreg in ldw_events:
                        if li <= gp or li >= i:
                            continue
                        if ins_ldw_mm.get(ln) == ins.name:
                            continue
                        if _overlap(lreg, reg):
                            clean = False
                            break
                    if clean:
                        drop.add(ins.name)
                n_drop += len(drop)
                new = []
                i, n = 0, len(insts)
                while i < n:
                    ins = insts[i]
                    mmn = ins_ldw_mm.get(getattr(ins, "name", None))
                    if mmn is not None:
                        si = ins.sync_info
                        syncfree = si is None or (not si.on_wait and not si.on_update)
                        if mmn in drop and syncfree:
                            i += 1
                            continue
                        kept_ldw[mmn] = ins.name
                        n_keep += 1
                    new.append(ins)
                    i += 1
                out[bb] = new
            print(
                f"[kernel] group-LDW: {n_drop} per-MM loads dropped, "
                f"{n_keep} kept",
                file=sys.stderr,
            )
            return out

        return wrapped

    xT_d = nc.declare_dram_parameter("xT", [D, T], bf16, isOutput=False)
    w_d = {}
    for name in ("wq1", "wk1", "wv1", "wq2", "wk2", "wv2", "wo"):
        w_d[name] = nc.declare_dram_parameter(name, [D, D], bf16, isOutput=False)
    b_d = {}
    for name in ("bq1", "bk1", "bv1", "bq2", "bk2", "bv2", "bo"):
        b_d[name] = nc.declare_dram_parameter(name, [128, 4], f32, isOutput=False)
    out_d = nc.declare_dram_parameter("out", [D, T], f32, isOutput=True)

    _orig_legalize = _tile_mod.tile_legalize
    _tile_mod.tile_legalize = _legalize_filter(_orig_legalize)
    with TileContext(nc) as tc:
        with (
            tc.tile_pool(name="wpool", bufs=1) as wpool,
            tc.tile_pool(name="bpool", bufs=1) as bpool,
            tc.tile_pool(name="y1pool", bufs=1) as y1pool,
            tc.tile_pool(name="sbx", bufs=3) as sbx,
            tc.tile_pool(name="sbqk", bufs=3) as sbqk,
            tc.tile_pool(name="sbv", bufs=2) as sbv,
            tc.tile_pool(name="sba", bufs=2) as sba,
            tc.tile_pool(name="sbo", bufs=3) as sbo,
            tc.tile_pool(name="pp", bufs=4, space="PSUM") as pp,
            tc.tile_pool(name="ppy", bufs=1, space="PSUM") as ppy,
        ):
            # DMA issue queues (only sync/scalar/gpsimd can issue DMAs;
            # gpsimd is reserved for the t1->t2 scatters in steady state)
            qeng = [nc.sync, nc.scalar, nc.gpsimd]

            wt = {}
            bt = {}

            def load_w(name, engines):
                tiles = []
                for k in range(4):
                    t_ = wpool.tile([128, D], bf16, tag=f"{name}_{k}", name=f"w_{name}_{k}")
                    engines[k % len(engines)].dma_start(
                        out=t_, in_=w_d[name][128 * k : 128 * (k + 1), :]
                    )
                    tiles.append(t_)
                wt[name] = tiles

            def load_b(name, engine):
                t_ = bpool.tile([128, 4], f32, tag=name, name=f"b_{name}")
                engine.dma_start(out=t_, in_=b_d[name][:, :])
                bt[name] = t_

            def prefetch_x(i, engines):
                cols = slice(CH * i, CH * (i + 1))
                xc = []
                for k in range(4):
                    t_ = sbx.tile([128, CH], bf16, tag=f"xc{k}", name=f"xc{k}")
                    engines[k % len(engines)].dma_start(
                        out=t_, in_=xT_d[128 * k : 128 * (k + 1), cols]
                    )
                    xc.append(t_)
                return xc

            # stage-1 output, feature-major [D, T] in t2 order
            # (col = p*1024 + gg*32 + ng)
            y1 = [y1pool.tile([128, T], bf16, tag=f"y1_{r}", name=f"y1_{r}") for r in range(4)]
            # scatter view: col = ((pp*32 + g)*2 + u)*16 + sl
            y1sc = [
                y1[k].rearrange("p (pp g u sl) -> p pp u g sl", pp=8, g=32, u=2)
                for k in range(4)
            ]

            def lin_m(w_tiles, bias_tile, rhs_aps, out_tiles, m, on_vector=False):
                """One m-tile of a feature-major linear: out[m] = bias +
                (W^T @ x)[128m:128m+128, :]."""
                ps = pp.tile([128, CH], f32, tag="ps", name="ps_lin")
                for k in range(4):
                    nc.tensor.matmul(
                        ps,
                        lhsT=w_tiles[k][:, 128 * m : 128 * (m + 1)],
                        rhs=rhs_aps[k],
                        start=(k == 0),
                        stop=(k == 3),
                    )
                if on_vector:
                    nc.vector.tensor_scalar_add(
                        out_tiles[m], ps, bias_tile[:, m : m + 1]
                    )
                else:
                    nc.scalar.activation(
                        out=out_tiles[m],
                        in_=ps,
                        func=AF.Identity,
                        bias=bias_tile[:, m : m + 1],
                    )

            def vlin_j(w_tiles, lhsT_aps, out_tiles, j):
                """One token-group of the v-linear: out[j] = [128 tokens,
                512 dims] (token-major), no bias."""
                ps = pp.tile([128, D], f32, tag="ps", name="ps_vlin")
                for k in range(4):
                    nc.tensor.matmul(
                        ps,
                        lhsT=lhsT_aps[j][k],
                        rhs=w_tiles[k],
                        start=(k == 0),
                        stop=(k == 3),
                    )
                if j % 2 == 0:
                    nc.scalar.activation(
                        out=out_tiles[j], in_=ps, func=AF.Identity
                    )
                else:
                    nc.vector.tensor_copy(out_tiles[j], ps)

            def scores_quarter(qt, kt, ps_s, m):
                """Scores for head-pair m: 32 of the chunk's 128 problems.

                PSUM bank index always equals the PE row-tile index so that
                concurrently-running row tiles never write the same bank.
                scores placement: bank=h%2 (= row tile 64*(h%2)), strip=sl%4,
                colblk=(h//2)*4 + sl//4.
                """

                def s_mm(m, s, par, c, ldw_name):
                    r = nc.tensor.matmul(
                        ps_s[par][32 * c : 32 * c + 32,
                                  32 * (4 * m + s) : 32 * (4 * m + s) + 32],
                        lhsT=qt[m][64 * par : 64 * par + 64,
                                   128 * s + 32 * c : 128 * s + 32 * c + 32],
                        rhs=kt[m][64 * par : 64 * par + 64,
                                  32 * (4 * s + c) : 32 * (4 * s + c) + 32],
                        start=True,
                        stop=True,
                        tile_position=(64 * par, 32 * c),
                    )
                    if ldw_name is not None:
                        grp_mm[_ins(r).name] = (
                            ldw_name,
                            (64 * par, 64 * par + 64, 32 * c, 32 * c + 32),
                        )

                if GROUP_S == 1:
                    for s in range(4):
                        for c in range(4):
                            for par in range(2):
                                s_mm(m, s, par, c, None)
                else:  # GROUP_S == 4: paired half-array loads, then MMs
                    for s in range(4):
                        lws = []
                        for par in range(2):
                            lw = nc.tensor.ldweights(
                                qt[m][64 * par : 64 * par + 64,
                                      128 * s : 128 * (s + 1)],
                                tile_position=(64 * par, 0),
                            )
                            grp_ldw[_ins(lw).name] = (
                                64 * par, 64 * par + 64, 0, 128,
                            )
                            lws.append(_ins(lw).name)
                        for par in range(2):
                            for c in range(4):
                                s_mm(m, s, par, c, lws[par])

            def softmax_emit(ps_s):
                """Softmax over s (free dim); fused across both halves where
                possible; returns transposed normalized-A tiles for AV."""
                a_f = sba.tile([128, 2 * CH], f32, tag="a_f", name="a_f")
                sums = sba.tile([128, 32], f32, tag="sums")
                rs = sba.tile([128, 32], f32, tag="rs")
                a_n = sba.tile([128, 2 * CH], bf16, tag="a_n", name="a_n")
                a_t = [sba.tile([128, CH], bf16, tag=f"at{i}", name=f"a_t{i}") for i in range(2)]
                for sb in range(2):
                    nc.scalar.activation(
                        out=a_f[:, CH * sb : CH * (sb + 1)], in_=ps_s[sb], func=AF.Exp
                    )
                nc.vector.tensor_reduce(
                    out=sums,
                    in_=a_f.rearrange("p (j s) -> p j s", s=32),
                    axis=AX.X,
                    op=ALU.add,
                )
                nc.vector.reciprocal(rs, sums)
                rs_b = bass.AP(
                    tensor=rs.tensor,
                    offset=rs.offset,
                    ap=[*rs.ap, [0, 32]],
                )
                nc.vector.tensor_mul(
                    a_n.rearrange("p (j s) -> p j s", s=32),
                    a_f.rearrange("p (j s) -> p j s", s=32),
                    rs_b,
                )
                for sb in range(2):
                    nc.vector.transpose(a_t[sb], a_n[:, CH * sb : CH * (sb + 1)])
                return a_t

            def av_quarter(vt, a_t, ys, j):
                """AV for seq-quarter j: yT[e, l] blocks.  ys is one 4-bank
                tile; bank (col 512c) = row tile 32c holds the 32 problems
                with sl%4 == c; within a bank: partition rows 64*(h%2) = head
                parity, col block 32*((h//2)*4 + sl//4)."""

                def av_mm(j, hh, c, par, ldw_name):
                    cb = 512 * c + 32 * (4 * hh + j)
                    r = nc.tensor.matmul(
                        ys[64 * par : 64 * par + 64, cb : cb + 32],
                        lhsT=vt[j][32 * c : 32 * c + 32,
                                   128 * hh + 64 * par : 128 * hh + 64 * par + 64],
                        rhs=a_t[par][32 * c : 32 * c + 32,
                                     32 * (4 * hh + j) : 32 * (4 * hh + j) + 32],
                        start=True,
                        stop=True,
                        tile_position=(32 * c, 64 * par),
                    )
                    if ldw_name is not None:
                        grp_mm[_ins(r).name] = (
                            ldw_name,
                            (32 * c, 32 * c + 32, 64 * par, 64 * par + 64),
                        )

                if GROUP_AV == 1:
                    for hh in range(4):
                        for c in range(4):
                            for par in range(2):
                                av_mm(j, hh, c, par, None)
                elif GROUP_AV == 4:
                    for hh in range(4):
                        lws = []
                        for q2 in range(2):
                            lw = nc.tensor.ldweights(
                                vt[j][64 * q2 : 64 * q2 + 64,
                                      128 * hh : 128 * (hh + 1)],
                                tile_position=(64 * q2, 0),
                            )
                            grp_ldw[_ins(lw).name] = (
                                64 * q2, 64 * q2 + 64, 0, 128,
                            )
                            lws.append(_ins(lw).name)
                        for q2 in range(2):
                            for c in (2 * q2, 2 * q2 + 1):
                                for par in range(2):
                                    av_mm(j, hh, c, par, lws[q2])
                else:  # GROUP_AV == 2: one 32-row strip per LDW
                    for hh in range(4):
                        for c in range(4):
                            lw = nc.tensor.ldweights(
                                vt[j][32 * c : 32 * c + 32,
                                      128 * hh : 128 * (hh + 1)],
                                tile_position=(32 * c, 0),
                            )
                            grp_ldw[_ins(lw).name] = (
                                32 * c, 32 * c + 32, 0, 128,
                            )
                            for par in range(2):
                                av_mm(j, hh, c, par, _ins(lw).name)

            def av_evacs(ys, dsts, vbias_tile):
                """Evacuate the whole 4-bank AV psum in 4 contiguous ops:
                op hh reads [j, r, g] (strides 32, 512, 1) and writes
                dsts[hh] cols 0..511 sequentially (col = 32*sl + g,
                sl = 4j+r)."""
                ysv = ys.rearrange(
                    "p (r hh j g) -> p hh j r g", r=4, hh=4, j=4, g=32
                )
                for hh in range(4):
                    src = ysv[:, hh]
                    dstv = dsts[hh].rearrange("p (j r g) -> p j r g", j=4, r=4)
                    if hh % 2 == 1:
                        nc.vector.tensor_scalar_add(
                            dstv, src, vbias_tile[:, hh : hh + 1]
                        )
                    else:
                        nc.scalar.activation(
                            out=dstv,
                            in_=src,
                            func=AF.Identity,
                            bias=vbias_tile[:, hh : hh + 1],
                        )

            def produce_qk(i, xc=None):
                """q/k linears for chunk i (i<16: stage-1, else stage-2) with
                the chunk's scores quarters interleaved per m-tile, so the PE
                alternates dense linear streams and small-matmul bursts."""
                qt = [sbqk.tile([128, CH], bf16, tag=f"qt{m}", name=f"qt{m}") for m in range(4)]
                kt = [sbqk.tile([128, CH], bf16, tag=f"kt{m}", name=f"kt{m}") for m in range(4)]
                vt = [sbv.tile([128, D], bf16, tag=f"vt{j}", name=f"vt{j}") for j in range(4)]
                ps_s = [pp.tile([128, CH], f32, tag="ps", name="ps_s") for _ in range(2)]
                if i < NCHUNK:
                    rhs_aps = [x[:, :] for x in xc]
                    lhsT_aps = [
                        [xc[k][:, 128 * j : 128 * (j + 1)] for k in range(4)]
                        for j in range(4)
                    ]
                    wq, bq, wk, bk, wv, vb = "wq1", "bq1", "wk1", "bk1", "wv1", "bv1"
                    stage2 = False
                else:
                    c2 = i - NCHUNK
                    cols = slice(CH * c2, CH * (c2 + 1))
                    rhs_aps = [y1[k][:, cols] for k in range(4)]
                    lhsT_aps = [
                        [
                            y1[k][:, CH * c2 + 128 * j : CH * c2 + 128 * (j + 1)]
                            for k in range(4)
                        ]
                        for j in range(4)
                    ]
                    wq, bq, wk, bk, wv, vb = "wq2", "bq2", "wk2", "bk2", "wv2", "bv2"
                    stage2 = True
                # scores quarter m is emitted one m-tile late so its q/k
                # evacuations (scalar q / vector k) have a full linear
                # m-tile of slack to complete before the PE reaches it
                for m in range(4):
                    lin_m(wt[wq], bt[bq], rhs_aps, qt, m)
                    lin_m(wt[wk], bt[bk], rhs_aps, kt, m, on_vector=True)
                    if m > 0:
                        scores_quarter(qt, kt, ps_s, m - 1)
                scores_quarter(qt, kt, ps_s, 3)
                return dict(qt=qt, kt=kt, vt=vt, ps_s=ps_s, lhsT_aps=lhsT_aps,
                            wv=wv, vb=bt[vb], stage2=stage2, c=i,
                            a_t=None, ys=None, tmp=None)

            def produce_v_j(st_, j):
                vlin_j(wt[st_["wv"]], st_["lhsT_aps"], st_["vt"], j)

            def av_q(st_, j):
                if st_["ys"] is None:
                    st_["ys"] = ppy.tile([128, 4 * CH], f32, tag="ys", name="ps_y4")
                av_quarter(st_["vt"], st_["a_t"], st_["ys"], j)

            def evac_chunk(st_):
                c = st_["c"]
                st_["tmp"] = [
                    sbo.tile([128, CH], bf16, tag=f"y2_{r}", name=f"y2_{r}")
                    for r in range(4)
                ]
                av_evacs(st_["ys"], st_["tmp"], st_["vb"])
                if not st_["stage2"]:
                    # t1->t2 scatter on GpSimd: tmp col = 32*sl + g ->
                    # y1 col = 1024*(c//2) + g*32 + 16*(c%2) + sl.
                    # Iteration (g outer, sl inner): dst writes land in
                    # 16-element contiguous runs.
                    for k in range(4):
                        dst = y1sc[k][:, c // 2 : c // 2 + 1,
                                      c % 2 : c % 2 + 1, :, :]
                        src = st_["tmp"][k].rearrange("p (sl g) -> p g sl", g=32)
                        nc.gpsimd.tensor_copy(dst, src)

            def out_linear_m(c2, y2t, m, final=False):
                ps = pp.tile([128, CH], f32, tag="ps", name="ps_lin")
                for k in range(4):
                    nc.tensor.matmul(
                        ps,
                        lhsT=wt["wo"][k][:, 128 * m : 128 * (m + 1)],
                        rhs=y2t[k],
                        start=(k == 0),
                        stop=(k == 3),
                    )
                os_ = sbo.tile([128, CH], f32, tag=f"os{m}", name=f"os{m}")
                if final and m % 2 == 1:
                    nc.vector.tensor_scalar_add(os_, ps, bt["bo"][:, m : m + 1])
                else:
                    nc.scalar.activation(
                        out=os_, in_=ps, func=AF.Identity,
                        bias=bt["bo"][:, m : m + 1],
                    )
                eng = nc.scalar if final and m % 2 == 1 else nc.sync
                eng.dma_start(
                    out=out_d[128 * m : 128 * (m + 1), CH * c2 : CH * (c2 + 1)],
                    in_=os_,
                )

            def out_linear(c2, y2t, final=False):
                for m in range(4):
                    out_linear_m(c2, y2t, m, final=final)

            # ---- prologue: spread initial loads across the DMA queues so
            # the first q-linear can start ASAP.
            load_w("wq1", qeng)
            xcache = {0: prefetch_x(0, qeng)}
            load_b("bq1", nc.sync)
            load_b("bk1", nc.scalar)
            load_w("wk1", qeng)
            load_b("bv1", nc.gpsimd)
            load_w("wv1", qeng)
            xcache[1] = prefetch_x(1, qeng)
            xcache[2] = prefetch_x(2, qeng)

            st = {ORDER[0]: produce_qk(ORDER[0], xc=xcache.pop(0))}
            # stage-2 / out weights load behind the critical path (NOT on
            # gpsimd: it runs the scatters)
            for name, eng in (("wq2", nc.sync), ("wk2", nc.scalar),
                              ("wv2", nc.sync), ("wo", nc.scalar)):
                load_w(name, [eng])
            for name in ("bq2", "bk2", "bv2", "bo"):
                load_b(name, nc.sync)
            st[ORDER[0]]["a_t"] = softmax_emit(st[ORDER[0]]["ps_s"])
            for j in range(4):
                produce_v_j(st[ORDER[0]], j)

            # x-prefetch plan: at position P, prefetch the stage-1 chunk
            # appearing at position P+2 (bufs=3 ring)
            s1_at = {P: c for P, c in enumerate(ORDER) if c < NCHUNK}

            out_pend = None
            NSTEP = len(ORDER)
            for P in range(NSTEP):
                cur = st.pop(ORDER[P])
                nxt = None
                last = P == NSTEP - 1
                if P + 2 in s1_at and s1_at[P + 2] not in (0, 1, 2):
                    xcache[s1_at[P + 2]] = prefetch_x(s1_at[P + 2], [nc.sync])
                if not last:
                    c_n = ORDER[P + 1]
                    nxt = produce_qk(c_n, xc=xcache.pop(c_n, None))
                if out_pend is not None and not last:
                    out_linear(*out_pend)
                    out_pend = None
                if nxt is not None:
                    nxt["a_t"] = softmax_emit(nxt["ps_s"])
                    for j in range(4):
                        produce_v_j(nxt, j)
                        av_q(cur, j)
                else:
                    # no next chunk to interleave: fill the attention burst
                    # with the pending out-linear's m-tiles instead
                    po = out_pend
                    out_pend = None
                    for j in range(4):
                        if po is not None:
                            out_linear_m(po[0], po[1], j, final=True)
                        av_q(cur, j)
                evac_chunk(cur)
                if cur["stage2"]:
                    out_pend = (cur["c"] - NCHUNK, cur["tmp"])
                if nxt is not None:
                    st[ORDER[P + 1]] = nxt
            out_linear(*out_pend, final=True)
    _tile_mod.tile_legalize = _orig_legalize
    nc.finalize()
    _verify_groups(nc, grp_mm, grp_ldw, kept_ldw)
    return nc


def _verify_groups(nc, grp_mm, grp_ldw, kept_ldw):
    """Walk the final instruction stream and assert that every grouped
    matmul's most recent region-overlapping LDWEIGHTS is exactly the load
    it expects (its group load, or its own kept per-MM load)."""
    from concourse import mybir

    if not grp_mm:
        return

    def ldw_region(ins):
        if ins.name in grp_ldw:
            return grp_ldw[ins.name]
        tp = ins.tile_position or (0, 0)
        ts = ins.tile_size or (128, 128)
        return (tp[0], tp[0] + ts[0], tp[1], tp[1] + ts[1])

    ldws = []
    checked = 0
    for bb in nc.m.functions[0].blocks:
        for ins in bb.instructions:
            if isinstance(ins, mybir.InstLdweights):
                ldws.append((ins.name, ldw_region(ins)))
            elif isinstance(ins, mybir.InstMatmult) and ins.name in grp_mm:
                want, reg = grp_mm[ins.name]
                want = kept_ldw.get(ins.name, want)
                got = None
                for n, lreg in reversed(ldws):
                    if (lreg[0] < reg[1] and reg[0] < lreg[1]
                            and lreg[2] < reg[3] and reg[2] < lreg[3]):
                        got = n
                        break
                if got != want:
                    raise RuntimeError(
                        f"group-LDW violation: matmul {ins.name} region "
                        f"{reg} expects weights from {want} but last "
                        f"overlapping LDW is {got}"
                    )
                checked += 1
    if checked != len(grp_mm):
        raise RuntimeError(
            f"group-LDW verify: saw {checked} of {len(grp_mm)} grouped matmuls"
        )


def _get_graph():
    global _GRAPH
    if _GRAPH is None:
        _GRAPH = _build_graph()
    return _GRAPH


def _host_pack(x, q1_w, q1_b, k1_w, k1_b, v1_w, v1_b, q2_w, q2_b, k2_w, k2_b,
               v2_w, v2_b, out_w, out_b):
    import ml_dtypes

    bf = ml_dtypes.bfloat16
    scale = 1.0 / np.sqrt(E)

    def wT(w, s=1.0):
        return np.ascontiguousarray((w * s).astype(np.float32).T).astype(bf)

    def bia(b, s=1.0):
        return np.ascontiguousarray((b * s).astype(np.float32).reshape(4, 128).T)

    common = {
        "wq1": wT(q1_w, scale), "wk1": wT(k1_w), "wv1": wT(v1_w),
        "wq2": wT(q2_w, scale), "wk2": wT(k2_w), "wv2": wT(v2_w),
        "wo": wT(out_w),
        "bq1": bia(q1_b, scale), "bk1": bia(k1_b), "bv1": bia(v1_b),
        "bq2": bia(q2_b, scale), "bk2": bia(k2_b), "bv2": bia(v2_b),
        "bo": bia(out_b),
    }
    in_maps = []
    for b in range(B):
        # x[b]: [1024(ch=ng*32+gg), 8(p), 512] -> token t = p*1024 + ch
        xb = np.asarray(x[b]).transpose(1, 0, 2).reshape(T, D)
        xT = np.ascontiguousarray(xb.T).astype(bf)
        m = dict(common)
        m["xT"] = xT
        in_maps.append(m)
    return in_maps


def _host_unpack(results):
    # device out: [512, 8192] f32, col t2 = p*1024 + gg*32 + ng
    ng_, gg_, p_ = np.meshgrid(
        np.arange(NUM_G), np.arange(G), np.arange(NUM_P), indexing="ij"
    )
    idx = p_ * 1024 + gg_ * 32 + ng_
    out = np.empty((B, NUM_G * G, NUM_P, D), dtype=np.float32)
    for b in range(B):
        y = results[b]["out"].T  # [8192, 512]
        out[b] = y[idx].reshape(NUM_G * G, NUM_P, D)
    return out


def kernel(**inputs):
    global LAST_EXEC_TIME_NS, LAST_TRACE
    from concourse.bass_utils import run_bass_kernel_spmd

    nc = _get_graph()
    in_maps = _host_pack(**inputs)
    trace = os.environ.get("KBENCH_TRACE") == "1"
    res = run_bass_kernel_spmd(nc, in_maps, list(range(8)), trace=trace)
    LAST_EXEC_TIME_NS = res.exec_time_ns
    it = res.instructions_and_trace
    LAST_TRACE = it[1] if it else None
    return _host_unpack(res.results)
